# revision 8
# baseline (speedup 1.0000x reference)
"""GQA attention (dense_transformer) TRN2 Bass kernel — 8 NeuronCores.

Problem: b=2, s=2048, d=2048, nh=16, nkv=4, hd=128, causal GQA attention
block with RMS-normed+RoPE'd q/k and per-head q gains.

Sharding: batch DP=2 x head TP=4  ->  8 cores. Each core handles one batch
element, 4 q heads, 1 kv head. Wq/Wk/Wv column-sharded, Wo row-sharded;
partial outputs summed on host.

v3 dataflow per core (matmuls bf16 in / fp32 PSUM):
  1. Block-0 projections run dc-major: the four 128-row s-tiles accumulate
     in four parallel PSUM tiles while the wq/xb0 DMA waves stream in, so
     the PE starts ~7us in and tracks the DMA. Later s-tiles are st-major
     as before. RMS stats + bit-trick rsqrt run on GpSimd (otherwise
     idle); per-head q gains fold into the exp scale (qg/sqrt(hd) as a
     per-partition scale AP), so the rinv chain is gain-free. Rope
     scalar_tensor_tensors split between DVE and GpSimd.
  2. Attention per (q-chunk, head): scores S^T = K-tile @ Q-chunk in
     kt-PAIRS into a 2-bank PSUM tile so one Exp covers 1024 columns;
     causal masking multiplies only the 128-wide diagonal block; A@V per
     128-q-subtile with pt stationary and [V | 1] moving so the softmax
     denominator rides along as PSUM column 128.
  3. Output projection interleaved between attention heads; partial outT
     written as bf16 (summed in f32 on host); the PSUM->SBUF evacuations
     alternate engines in the tail drain.
"""

import math
import sys

if "/opt/trn_rl_repo" not in sys.path:
    sys.path.insert(0, "/opt/trn_rl_repo")

import numpy as np
import ml_dtypes

import concourse.mybir as mybir
import concourse.tile as tile
from concourse.bass_types import AP
from concourse import bacc
from concourse.bass_utils import run_bass_kernel_spmd

F32 = mybir.dt.float32
I32 = mybir.dt.int32
BF16 = mybir.dt.bfloat16
AF = mybir.ActivationFunctionType
ALU = mybir.AluOpType
AXL = mybir.AxisListType

BF16NP = ml_dtypes.bfloat16
RMS_EPS = float(np.finfo(np.float32).eps)

S, D, NQ, HD = 2048, 2048, 4, 128
DQ = NQ * HD            # 512: per-core q width
NTP = 4                 # tensor-parallel ways (heads)
NB = 2                  # batch (data-parallel ways)
NCORES = 8

_NC_CACHE = {}


def build_kernel(S=S, D=D, NQ=NQ, HD=HD, num_devices=NCORES):
    DQ = NQ * HD
    NST = S // 128          # s-tiles
    NDC = D // 128          # d-chunks (projection contraction)
    NQC = S // 512          # q-chunks for attention
    NBLK = 4                # xT streaming blocks (512 s-cols each)
    H = HD // 2
    NH1 = NQ + 1
    # offsets of the four [128,129] AV accumulators inside the 2-bank pyp
    # tile (16B aligned, none crossing a PSUM bank boundary); transposed
    # output parks at [644:900) as bf16.
    PYOFF = (0, 144, 288, 512)

    nc = bacc.Bacc("TRN2", target_bir_lowering=False, debug=False,
                   num_devices=num_devices)

    xT = nc.dram_tensor("xT", [D, S], BF16, kind="ExternalInput").ap()
    wq = nc.dram_tensor("wq", [D, DQ], BF16, kind="ExternalInput").ap()
    wkv = nc.dram_tensor("wkv", [D, 2 * HD], BF16, kind="ExternalInput").ap()
    wo = nc.dram_tensor("wo", [DQ, D], BF16, kind="ExternalInput").ap()
    cst = nc.dram_tensor("cst", [S, 2 * HD], BF16, kind="ExternalInput").ap()
    # per-head exp scales qg[h]/sqrt(hd), broadcast over partitions
    qgb = nc.dram_tensor("qgb", [128, NQ], F32, kind="ExternalInput").ap()
    ident = nc.dram_tensor("ident", [128, 128], BF16, kind="ExternalInput").ap()
    msk = nc.dram_tensor("msk", [128, 128], BF16, kind="ExternalInput").ap()
    ones = nc.dram_tensor("ones", [128, 128], BF16, kind="ExternalInput").ap()
    outT = nc.dram_tensor("outT", [D, S], BF16, kind="ExternalOutput").ap()

    with tile.TileContext(nc) as tc:
        from contextlib import ExitStack
        with ExitStack() as ctx:
            consts = ctx.enter_context(tc.tile_pool(name="consts", bufs=1))
            wpool = ctx.enter_context(tc.tile_pool(name="w", bufs=1))
            xpool = ctx.enter_context(tc.tile_pool(name="xT", bufs=2))
            qt_pool = ctx.enter_context(tc.tile_pool(name="qt", bufs=1))
            yt_pool = ctx.enter_context(tc.tile_pool(name="yt", bufs=1))
            v_pool = ctx.enter_context(tc.tile_pool(name="vrow", bufs=1))
            sq_pool = ctx.enter_context(tc.tile_pool(name="sq", bufs=3))
            st_pool = ctx.enter_context(tc.tile_pool(name="stat", bufs=3))
            tv_pool = ctx.enter_context(tc.tile_pool(name="tv", bufs=2))
            ro_pool = ctx.enter_context(tc.tile_pool(name="ro", bufs=3))
            pt_pool = ctx.enter_context(tc.tile_pool(name="ptile", bufs=6))
            rn_pool = ctx.enter_context(tc.tile_pool(name="rn", bufs=3))
            ob_pool = ctx.enter_context(tc.tile_pool(name="ob", bufs=6))
            # PSUM: pA 2x[128,1024] (4 banks) + pB 1x[128,1024] (2 banks)
            # + pC 2x[128,512] (2 banks) = all 8 banks.
            pA = ctx.enter_context(tc.tile_pool(name="pA", bufs=2, space="PSUM"))
            pB = ctx.enter_context(tc.tile_pool(name="pB", bufs=1, space="PSUM"))
            pC = ctx.enter_context(tc.tile_pool(name="pC", bufs=2, space="PSUM"))

            xTr = xT.rearrange("(n p) m -> p n m", p=128)
            wqr = wq.rearrange("(n p) m -> p n m", p=128)
            wkvr = wkv.rearrange("(n p) m -> p n m", p=128)
            cstr = cst.rearrange("(n p) m -> p n m", p=128)

            wq_sb = wpool.tile([128, NDC, DQ], BF16, tag="wq")
            wkv_sb = wpool.tile([128, NDC, 2 * HD], BF16, tag="wkv")
            cst_sb = consts.tile([128, NST, 2 * HD], BF16, tag="cst")
            ident_sb = consts.tile([128, 128], BF16, tag="ident")
            qgb_sb = consts.tile([128, NQ], F32, tag="qgb")
            ones_sb = consts.tile([128, 128], BF16, tag="ones")
            msk_sb = consts.tile([128, 128], BF16, tag="msk")
            wo_sb = wpool.tile([128, NQ, D], BF16, tag="wo")

            x_blocks = [None] * NBLK
            x_blocks[0] = xpool.tile([128, NDC, 512], BF16, name="xb0",
                                     tag="xb")
            x_blocks[1] = xpool.tile([128, NDC, 512], BF16, name="xb1",
                                     tag="xb")

            # ---- tiny consts + first cst rows on the scalar DMA queue
            # (parallel issue; negligible bandwidth steal), everything else
            # priority-ordered on the sync queue.
            nc.scalar.dma_start(ident_sb[:], ident)
            nc.scalar.dma_start(qgb_sb[:], qgb)
            nc.scalar.dma_start(ones_sb[:], ones)
            nc.scalar.dma_start(msk_sb[:], msk)
            nc.scalar.dma_start(cst_sb[:, 0:4, :], cstr[:, 0:4, :])

            # sync queue: wq/xb0 interleaved 2-dc waves feed the dc-major
            # phase-A chains; then wkv, xb1, rest of cst, xb2/xb3 (issued
            # later), wo.
            for w in range(NDC // 2):
                nc.sync.dma_start(wq_sb[:, 2 * w:2 * w + 2, :],
                                  wqr[:, 2 * w:2 * w + 2, :])
                nc.sync.dma_start(x_blocks[0][:, 2 * w:2 * w + 2, :],
                                  xTr[:, 2 * w:2 * w + 2, 0:512])
            for w in range(4):
                nc.sync.dma_start(wkv_sb[:, 4 * w:4 * w + 4, :],
                                  wkvr[:, 4 * w:4 * w + 4, :])
            for w in range(4):
                nc.sync.dma_start(x_blocks[1][:, 4 * w:4 * w + 4, :],
                                  xTr[:, 4 * w:4 * w + 4, 512:1024])
            nc.sync.dma_start(cst_sb[:, 4:NST, :], cstr[:, 4:NST, :])

            # HAM warmup: PE work with NO input dependency (reads an
            # uninitialized SBUF scratch tile) so the clock gate is at 8/8
            # and the PE pipeline primed when the first real matmuls arrive.
            wsrc = consts.tile([128, 128], BF16, tag="wsrc")
            nc.vector.memset(wsrc[:], 1.0)
            warm = pC.tile([128, 512], F32, name="warm", tag="c")
            for i in range(55):
                nc.tensor.matmul(warm[:, 0:128], wsrc[:], wsrc[:],
                                 start=True, stop=True)

            qt_all = qt_pool.tile([128, NH1, S], BF16, name="qt_all",
                                  tag="qt_all")
            yt_tiles = [yt_pool.tile([128, S], BF16, name=f"yt{h}", tag=f"yt{h}")
                        for h in range(NQ)]
            v_tiles = [v_pool.tile([128, 132], BF16, name=f"v{st}", tag=f"v{st}")
                       for st in range(NST)]

            # ---- Phase 1: projections + rms-norm + rope + transpose ----
            # The PE transposes of s-tile st are deferred until the next
            # s-tile's projection matmuls have been emitted, so the rope
            # chain has a full tile of slack before the PE needs its output.
            tr_state = {"pend": []}

            def flush_one_tr():
                st, ro5 = tr_state["pend"].pop(0)
                bt = pB.tile([128, 1024], F32, name="bt", tag="b")
                ptv = bt[:, 0:NH1 * 64].bitcast(BF16)  # [128, 640] bf16
                for i in range(NH1):
                    nc.tensor.transpose(ptv[:, i * 128:(i + 1) * 128],
                                        ro5[:, i * HD:(i + 1) * HD],
                                        ident_sb[:])
                nc.vector.tensor_copy(
                    qt_all[:, :, st * 128:(st + 1) * 128],
                    ptv.rearrange("p (h c) -> p h c", c=128))

            def flush_tr():
                while tr_state["pend"]:
                    flush_one_tr()

            def post_stile(st, pq, pkv):
                """RMS stats off PSUM, rsqrt chain, scaled evac, rope."""
                # V row tile [v | 1] straight from PSUM
                nc.vector.tensor_copy(v_tiles[st][:, 0:HD],
                                      pkv[:, HD:2 * HD])
                nc.vector.tensor_copy(v_tiles[st][:, HD:HD + 1],
                                      ones_sb[:, 0:1])

                # RMS sums read PSUM directly: Square+accum_out on ScalarE
                ssq = st_pool.tile([128, NH1], F32, tag="ssq")
                sqs = sq_pool.tile([128, HD], F32, tag="sqs")
                for i in range(NH1):
                    src = pq[:, i * HD:(i + 1) * HD] if i < NQ else \
                        pkv[:, 0:HD]
                    nc.scalar.activation(sqs[:], src, AF.Square,
                                         accum_out=ssq[:, i:i + 1])

                # rinv = (mean(q^2)+eps)**-0.5: float ops on GpSimd,
                # int bit-trick on DVE
                m = st_pool.tile([128, NH1], F32, tag="m")
                nc.gpsimd.tensor_scalar(m[:], ssq[:], 1.0 / HD, RMS_EPS,
                                        op0=ALU.mult, op1=ALU.add)
                y0 = st_pool.tile([128, NH1], F32, tag="y0")
                nc.vector.tensor_scalar(y0[:].bitcast(I32),
                                        m[:].bitcast(I32), 1, None,
                                        op0=ALU.arith_shift_right)
                nc.vector.tensor_scalar(y0[:].bitcast(I32),
                                        y0[:].bitcast(I32),
                                        -1, 0x5F3759DF,
                                        op0=ALU.mult, op1=ALU.add)
                rinv = y0
                aa = st_pool.tile([128, NH1], F32, tag="nr_a")
                nc.gpsimd.tensor_mul(aa[:], rinv[:], rinv[:])
                nc.gpsimd.tensor_mul(aa[:], aa[:], m[:])
                nc.gpsimd.tensor_scalar(aa[:], aa[:], -0.5, 1.5,
                                        op0=ALU.mult, op1=ALU.add)
                nxt = st_pool.tile([128, NH1], F32, tag="nr_y")
                nc.gpsimd.tensor_mul(nxt[:], rinv[:], aa[:])
                rinv = nxt

                # evacuate PSUM with the norm applied: per-head copy with
                # per-partition scale=rinv (3 on ScalarE, 2 on DVE)
                qkv = sq_pool.tile([128, NH1 * HD], BF16, tag="qkv")
                with tc.high_priority():
                    for i in range(NH1):
                        src = pq[:, i * HD:(i + 1) * HD] if i < NQ else \
                            pkv[:, 0:HD]
                        dst = qkv[:, i * HD:(i + 1) * HD]
                        if i in (2, 3):
                            nc.vector.tensor_scalar(
                                dst, src, rinv[:, i:i + 1], None,
                                op0=ALU.mult)
                        else:
                            nc.scalar.activation(dst, src, AF.Copy,
                                                 scale=rinv[:, i:i + 1])

                # rope: per head [t|v] = qhat_rep * [c|c|-s|s]; plain
                # tensor_tensor split DVE / GpSimd so they run in parallel
                cst_t = cst_sb[:, st, :]
                tv5 = tv_pool.tile([128, NH1 * 2 * HD], BF16, tag="tv5")
                for i in range(NH1):
                    q_ap = qkv[:, i * HD:(i + 1) * HD]
                    q_rep = AP(q_ap.tensor, q_ap.offset,
                               [q_ap.ap[0], [0, 2], [1, HD]])
                    eng = nc.vector if i % 2 == 0 else nc.gpsimd
                    eng.tensor_mul(
                        tv5[:, i * 2 * HD:(i + 1) * 2 * HD], q_rep, cst_t)
                ro5 = ro_pool.tile([128, NH1 * HD], BF16, tag="ro5")
                b5 = tv5[:]
                t_view = AP(b5.tensor, b5.offset,
                            [b5.ap[0], [2 * HD, NH1], [H, 2], [1, H]])
                v_view = AP(b5.tensor, b5.offset + HD + H,
                            [b5.ap[0], [2 * HD, NH1], [-H, 2], [1, H]])
                r5 = ro5[:]
                o_view = AP(r5.tensor, r5.offset,
                            [r5.ap[0], [HD, NH1], [H, 2], [1, H]])
                nc.vector.tensor_add(o_view, t_view, v_view)
                tr_state["pend"].append((st, ro5))

            def process_stile(st, xb, st4):
                at = pA.tile([128, 1024], F32, name="at", tag="a")
                pq = at[:, 0:DQ]
                pkv = at[:, DQ:DQ + 2 * HD]
                for dc in range(NDC):
                    nc.tensor.matmul(pq, xb[:, dc, st4 * 128:(st4 + 1) * 128],
                                     wq_sb[:, dc, :],
                                     start=dc == 0, stop=dc == NDC - 1)
                for dc in range(NDC):
                    nc.tensor.matmul(pkv, xb[:, dc, st4 * 128:(st4 + 1) * 128],
                                     wkv_sb[:, dc, :],
                                     start=dc == 0, stop=dc == NDC - 1)
                if len(tr_state["pend"]) >= 2:
                    flush_one_tr()
                post_stile(st, pq, pkv)

            # ---- Phase A/B: block-0 s-tiles 0-3 dc-major across four
            # parallel PSUM accumulators, tracking the wq/xb0 DMA waves.
            pa0 = pA.tile([128, 1024], F32, name="pa0", tag="a")
            pa1 = pA.tile([128, 1024], F32, name="pa1", tag="a")
            pb0 = pB.tile([128, 1024], F32, name="pb0", tag="b")
            pc0 = pC.tile([128, 512], F32, name="pc0", tag="c")
            pc1 = pC.tile([128, 512], F32, name="pc1", tag="c")
            pqs = [pa0[:, 0:DQ], pa1[:, 0:DQ], pb0[:, 0:DQ], pc0[:, 0:DQ]]
            pkvs = [pa0[:, DQ:DQ + 2 * HD], pa1[:, DQ:DQ + 2 * HD],
                    pb0[:, DQ:DQ + 2 * HD], pc1[:, 0:2 * HD]]
            xb0 = x_blocks[0]
            for dc in range(NDC):
                for st4 in range(4):
                    nc.tensor.matmul(
                        pqs[st4], xb0[:, dc, st4 * 128:(st4 + 1) * 128],
                        wq_sb[:, dc, :], start=dc == 0, stop=dc == NDC - 1,
                        skip_group_check=True)
            for dc in range(NDC):
                for st4 in range(4):
                    nc.tensor.matmul(
                        pkvs[st4], xb0[:, dc, st4 * 128:(st4 + 1) * 128],
                        wkv_sb[:, dc, :], start=dc == 0, stop=dc == NDC - 1,
                        skip_group_check=True)
            for st in range(4):
                post_stile(st, pqs[st], pkvs[st])

            # ---- Phases interleaved: projections block b -> attention
            # qc=b -> outproj qc=b-1 as PE filler between heads ----
            kt_row = qt_all[:, NQ, :]

            # out-projection dribbled one 128x512 tile at a time between
            # attention score groups (PE filler while ScalarE exps run);
            # outT DMA batched per 4 tiles.
            op_state = {"pending": [], "ob": None, "row": 0}

            def push_outproj(qcp):
                op_state["pending"].extend((qcp, dt) for dt in range(NST))

            def emit_outproj_unit(copy_eng="vector"):
                if not op_state["pending"]:
                    return
                qcp, dt = op_state["pending"].pop(0)
                if op_state["row"] == 0:
                    op_state["ob"] = ob_pool.tile([128, 4, 512], BF16,
                                                  name="ob", tag="ob")
                po = pC.tile([128, 512], F32, name="po", tag="c")
                for dqc in range(NQ):
                    nc.tensor.matmul(
                        po[:], wo_sb[:, dqc, dt * 128:(dt + 1) * 128],
                        yt_tiles[dqc][:, qcp * 512:(qcp + 1) * 512],
                        start=(dqc == 0), stop=(dqc == NQ - 1))
                if copy_eng == "scalar":
                    nc.scalar.copy(op_state["ob"][:, op_state["row"], :], po[:])
                else:
                    nc.vector.tensor_copy(
                        op_state["ob"][:, op_state["row"], :], po[:])
                op_state["row"] += 1
                if op_state["row"] == 4:
                    op_state["row"] = 0
                    nc.sync.dma_start(
                        outT[(dt - 3) * 128:(dt + 1) * 128,
                             qcp * 512:(qcp + 1) * 512].rearrange(
                                 "(n p) m -> p n m", p=128),
                        op_state["ob"][:])

            work_q = []  # pending s-tile closures (consumed as PE filler)

            def fill_slot():
                if work_q:
                    work_q.pop(0)()
                else:
                    emit_outproj_unit()

            def attention_head(qc, h):
                n_kt = 4 * qc + 4
                n_groups = n_kt // 2
                qs = qt_all[:, h, qc * 512:(qc + 1) * 512]
                sc_ap = qgb_sb[:, h:h + 1]

                def off_of(kt):
                    return max(0, kt - 4 * qc) * 128

                def emit_scores_group(g):
                    sp = pA.tile([128, 1024], F32, name="sp", tag="a")
                    ptp = pt_pool.tile([128, 2, 512], BF16, name="ptp",
                                       tag="ptp")
                    for u in (0, 1):
                        kt = 2 * g + u
                        off = off_of(kt)
                        nc.tensor.matmul(
                            sp[:, u * 512 + off:(u + 1) * 512],
                            kt_row[:, kt * 128:(kt + 1) * 128],
                            qs[:, off:512], start=True, stop=True)
                    if 2 * g + 1 < 4 * qc:  # both tiles non-diagonal
                        nc.scalar.activation(
                            ptp[:].rearrange("p a b -> p (a b)"),
                            sp[:], AF.Exp, scale=sc_ap)
                    else:
                        for u in (0, 1):
                            kt = 2 * g + u
                            off = off_of(kt)
                            nc.scalar.activation(ptp[:, u, off:512],
                                                 sp[:, u * 512 + off:(u + 1) * 512],
                                                 AF.Exp, scale=sc_ap)
                    for u in (0, 1):
                        kt = 2 * g + u
                        mdiag = kt - 4 * qc
                        if mdiag >= 0:
                            # only the 128-wide diagonal block needs masking
                            off = mdiag * 128
                            nc.vector.tensor_mul(
                                ptp[:, u, off:off + 128],
                                ptp[:, u, off:off + 128], msk_sb[:, 0:128])
                    return ptp

                def emit_av_group(g, ptp, pyp):
                    for u in (0, 1):
                        kt = 2 * g + u
                        j0 = max(0, kt - 4 * qc)
                        for j in range(j0, 4):
                            nc.tensor.matmul(
                                pyp[:, PYOFF[j]:PYOFF[j] + HD + 1],
                                ptp[:, u, j * 128:(j + 1) * 128],
                                v_tiles[kt][:, 0:HD + 1],
                                start=(kt == 0 and j in (0, 3)),
                                stop=(kt == 4 * qc + j),
                                skip_group_check=True)

                prev = emit_scores_group(0)
                fill_slot()
                # start=True on any matmul clears has_written for its whole
                # PSUM bank, which would wipe sibling accumulators sharing
                # the bank -- so zero the regions once and accumulate with
                # start=False throughout.
                pyp = pB.tile([128, 1024], F32, name="pyp", tag="b")
                for g in range(1, n_groups):
                    cur = emit_scores_group(g)
                    emit_av_group(g - 1, prev, pyp)
                    prev = cur
                    if qc < 2 or g % 2 == 1:
                        fill_slot()
                emit_av_group(n_groups - 1, prev, pyp)

                # normalize: rcp of the 4 denominator columns, then per-
                # partition scale of each [q,hd] block; transpose to [hd,q].
                rcp = rn_pool.tile([128, 4], F32, tag="rcp")
                r3 = AP(pyp.tensor, pyp.offset + HD,
                        [pyp.ap[0], [PYOFF[1] - PYOFF[0], 3]])
                nc.vector.reciprocal(out=rcp[:, 0:3], in_=r3)
                nc.vector.reciprocal(out=rcp[:, 3:4],
                                     in_=pyp[:, PYOFF[3] + HD:PYOFF[3] + HD + 1])
                yn = rn_pool.tile([128, NQ * HD], BF16, tag="yn")
                for j in range(4):
                    nc.vector.tensor_scalar(
                        yn[:, j * HD:(j + 1) * HD],
                        pyp[:, PYOFF[j]:PYOFF[j] + HD],
                        rcp[:, j:j + 1], None, op0=ALU.mult)
                ytr = pyp[:, 656:912].bitcast(BF16)  # [128, 512] bf16
                for j in range(4):
                    nc.tensor.transpose(ytr[:, j * 128:(j + 1) * 128],
                                        yn[:, j * HD:(j + 1) * HD],
                                        ident_sb[:])
                nc.vector.tensor_copy(
                    yt_tiles[h][:, qc * 512:(qc + 1) * 512], ytr)

            def prefetch_xb(b):
                x_blocks[b] = xpool.tile([128, NDC, 512], BF16,
                                         name=f"xb{b}", tag="xb")
                nc.sync.dma_start(x_blocks[b][:, 0:8, :],
                                  xTr[:, 0:8, b * 512:(b + 1) * 512])
                nc.sync.dma_start(x_blocks[b][:, 8:NDC, :],
                                  xTr[:, 8:NDC, b * 512:(b + 1) * 512])

            # stage A tail: s-tile 4 st-major (xb1 already streaming), so
            # the 1-deep transpose deferral never leaves a tile the next
            # stage's heads need pending at a stage boundary.
            process_stile(4, x_blocks[1], 0)

            # stages B..E: attention for qc in order; projection s-tiles of
            # later blocks and out-projection tiles serve as PE filler inside
            # the heads (supply matched to the heads' fill slots).
            STAGE_STILES = {0: range(5, 12), 1: range(12, 16),
                            2: range(0, 0), 3: range(0, 0)}
            for qc in range(NQC):
                if qc == 0:
                    prefetch_xb(2)
                    nc.sync.dma_start(
                        wo_sb[:], wo.rearrange("(n p) m -> p n m", p=128))
                    prefetch_xb(3)
                for st in STAGE_STILES[qc]:
                    work_q.append(
                        lambda st=st: process_stile(st, x_blocks[st // 4],
                                                    st % 4))
                flush_tr()  # heads of qc read qt up to st 4*qc+3
                for h in range(NQ):
                    attention_head(qc, h)
                while work_q:  # any unconsumed s-tiles must land before the
                    work_q.pop(0)()  # next stage's heads read their qt/v
                push_outproj(qc)
            di = 0
            while op_state["pending"]:
                emit_outproj_unit("scalar" if di % 2 else "vector")
                di += 1

    nc.compile()
    return nc


def get_nc():
    if "nc" not in _NC_CACHE:
        _NC_CACHE["nc"] = build_kernel()
    return _NC_CACHE["nc"]


def rope_tables(S=S, HD=HD):
    """Packed rope table [S, 2*HD]: [c | c | -s | s]."""
    f = 1.0 / (10000.0 ** (np.arange(0, HD, 2, dtype=np.float32) / HD))
    fr = np.outer(np.arange(S, dtype=np.float32), f)
    c = np.cos(fr).astype(np.float32)
    s = np.sin(fr).astype(np.float32)
    return np.concatenate([c, c, -s, s], axis=1)


def make_in_maps(x, Wq, Wk, Wv, Wo, qg):
    x = np.asarray(x, np.float32)
    Wq = np.asarray(Wq, np.float32)
    Wk = np.asarray(Wk, np.float32)
    Wv = np.asarray(Wv, np.float32)
    Wo = np.asarray(Wo, np.float32)
    qg = np.asarray(qg, np.float32)
    cst = rope_tables()
    ident = np.eye(128, dtype=BF16NP)
    ones = np.ones((128, 128), dtype=BF16NP)
    pp, ff = np.arange(128)[:, None], np.arange(128)[None, :]
    msk = (ff >= pp).astype(BF16NP)
    scale = np.float32(1.0 / np.sqrt(HD))
    xT = [np.ascontiguousarray(x[b].T).astype(BF16NP) for b in range(NB)]
    in_maps = []
    for b in range(NB):
        for tp in range(NTP):
            qgb_row = np.broadcast_to(
                (qg[tp * NQ:(tp + 1) * NQ] * scale)[None, :].astype(np.float32),
                (128, NQ)).copy()
            wkv = np.concatenate([
                Wk[tp * HD:(tp + 1) * HD, :].T,
                Wv[tp * HD:(tp + 1) * HD, :].T], axis=1)
            in_maps.append({
                "xT": xT[b],
                "wq": np.ascontiguousarray(
                    Wq[tp * DQ:(tp + 1) * DQ, :].T).astype(BF16NP),
                "wkv": np.ascontiguousarray(wkv).astype(BF16NP),
                "wo": np.ascontiguousarray(
                    Wo[:, tp * DQ:(tp + 1) * DQ].T).astype(BF16NP),
                "cst": cst.astype(BF16NP),
                "qgb": qgb_row,
                "ident": ident,
                "ones": ones,
                "msk": msk,
            })
    return in_maps


def run(x, Wq, Wk, Wv, Wo, qg, trace=False, **trace_kwargs):
    nc = get_nc()
    in_maps = make_in_maps(x, Wq, Wk, Wv, Wo, qg)
    res = run_bass_kernel_spmd(nc, in_maps, core_ids=list(range(NCORES)),
                               trace=trace, **trace_kwargs)
    out = np.empty((NB, S, D), np.float32)
    for b in range(NB):
        acc = res.results[b * NTP]["outT"].astype(np.float32)
        for tp in range(1, NTP):
            acc = acc + res.results[b * NTP + tp]["outT"].astype(np.float32)
        out[b] = acc.T
    return out, res


def kernel(x, Wq, Wk, Wv, Wo, qg):
    out, _ = run(x, Wq, Wk, Wv, Wo, qg)
    return out


# revision 11
# speedup vs baseline: 1.1106x; 1.1106x over previous
"""GQA attention (dense_transformer) TRN2 Bass kernel — 8 NeuronCores.

Problem: b=2, s=2048, d=2048, nh=16, nkv=4, hd=128, causal GQA attention
block with RMS-normed+RoPE'd q/k and per-head q gains.

Sharding: batch DP=2 x head TP=4  ->  8 cores. Each core handles one batch
element, 4 q heads, 1 kv head. Wq/Wk/Wv column-sharded, Wo row-sharded;
partial outputs summed on host.

v3 dataflow per core (matmuls bf16 in / fp32 PSUM):
  1. Block-0 projections run dc-major: the four 128-row s-tiles accumulate
     in four parallel PSUM tiles while the wq/xb0 DMA waves stream in, so
     the PE starts ~7us in and tracks the DMA. Later s-tiles are st-major
     as before. RMS stats + bit-trick rsqrt run on GpSimd (otherwise
     idle); per-head q gains fold into the exp scale (qg/sqrt(hd) as a
     per-partition scale AP), so the rinv chain is gain-free. Rope
     scalar_tensor_tensors split between DVE and GpSimd.
  2. Attention per (q-chunk, head): scores S^T = K-tile @ Q-chunk in
     kt-PAIRS into a 2-bank PSUM tile so one Exp covers 1024 columns;
     causal masking multiplies only the 128-wide diagonal block; A@V per
     128-q-subtile with pt stationary and [V | 1] moving so the softmax
     denominator rides along as PSUM column 128.
  3. Output projection interleaved between attention heads; partial outT
     written as bf16 (summed in f32 on host); the PSUM->SBUF evacuations
     alternate engines in the tail drain.
"""

import math
import sys

if "/opt/trn_rl_repo" not in sys.path:
    sys.path.insert(0, "/opt/trn_rl_repo")

import numpy as np
import ml_dtypes

import concourse.mybir as mybir
import concourse.tile as tile
from concourse.bass_types import AP
from concourse import bacc
from concourse.bass_utils import run_bass_kernel_spmd

F32 = mybir.dt.float32
I32 = mybir.dt.int32
BF16 = mybir.dt.bfloat16
AF = mybir.ActivationFunctionType
ALU = mybir.AluOpType
AXL = mybir.AxisListType

BF16NP = ml_dtypes.bfloat16
RMS_EPS = float(np.finfo(np.float32).eps)

S, D, NQ, HD = 2048, 2048, 4, 128
DQ = NQ * HD            # 512: per-core q width
NTP = 4                 # tensor-parallel ways (heads)
NB = 2                  # batch (data-parallel ways)
NCORES = 8

_NC_CACHE = {}


def build_kernel(S=S, D=D, NQ=NQ, HD=HD, num_devices=NCORES):
    DQ = NQ * HD
    NST = S // 128          # s-tiles
    NDC = D // 128          # d-chunks (projection contraction)
    NQC = S // 512          # q-chunks for attention
    NBLK = 4                # xT streaming blocks (512 s-cols each)
    H = HD // 2
    NH1 = NQ + 1
    # offsets of the four [128,129] AV accumulators inside the 2-bank pyp
    # tile (16B aligned, none crossing a PSUM bank boundary); transposed
    # output parks at [644:900) as bf16.
    PYOFF = (0, 144, 288, 512)

    nc = bacc.Bacc("TRN2", target_bir_lowering=False, debug=False,
                   num_devices=num_devices)

    xT = nc.dram_tensor("xT", [D, S], BF16, kind="ExternalInput").ap()
    wq = nc.dram_tensor("wq", [D, DQ], BF16, kind="ExternalInput").ap()
    wkv = nc.dram_tensor("wkv", [D, 2 * HD], BF16, kind="ExternalInput").ap()
    wo = nc.dram_tensor("wo", [DQ, D], BF16, kind="ExternalInput").ap()
    cst = nc.dram_tensor("cst", [S, 2 * HD], BF16, kind="ExternalInput").ap()
    # per-head exp scales qg[h]/sqrt(hd), broadcast over partitions
    qgb = nc.dram_tensor("qgb", [128, NQ], F32, kind="ExternalInput").ap()
    ident = nc.dram_tensor("ident", [128, 128], BF16, kind="ExternalInput").ap()
    msk = nc.dram_tensor("msk", [128, 128], BF16, kind="ExternalInput").ap()
    ones = nc.dram_tensor("ones", [128, 128], BF16, kind="ExternalInput").ap()
    outT = nc.dram_tensor("outT", [D, S], BF16, kind="ExternalOutput").ap()

    with tile.TileContext(nc) as tc:
        from contextlib import ExitStack
        with ExitStack() as ctx:
            consts = ctx.enter_context(tc.tile_pool(name="consts", bufs=1))
            wpool = ctx.enter_context(tc.tile_pool(name="w", bufs=1))
            xpool = ctx.enter_context(tc.tile_pool(name="xT", bufs=2))
            qt_pool = ctx.enter_context(tc.tile_pool(name="qt", bufs=1))
            yt_pool = ctx.enter_context(tc.tile_pool(name="yt", bufs=1))
            v_pool = ctx.enter_context(tc.tile_pool(name="vrow", bufs=1))
            sq_pool = ctx.enter_context(tc.tile_pool(name="sq", bufs=3))
            st_pool = ctx.enter_context(tc.tile_pool(name="stat", bufs=3))
            tv_pool = ctx.enter_context(tc.tile_pool(name="tv", bufs=2))
            ro_pool = ctx.enter_context(tc.tile_pool(name="ro", bufs=3))
            pt_pool = ctx.enter_context(tc.tile_pool(name="ptile", bufs=6))
            rn_pool = ctx.enter_context(tc.tile_pool(name="rn", bufs=3))
            ob_pool = ctx.enter_context(tc.tile_pool(name="ob", bufs=6))
            # PSUM: pA 2x[128,1024] (4 banks) + pB 1x[128,1024] (2 banks)
            # + pC 2x[128,512] (2 banks) = all 8 banks.
            pA = ctx.enter_context(tc.tile_pool(name="pA", bufs=2, space="PSUM"))
            pB = ctx.enter_context(tc.tile_pool(name="pB", bufs=1, space="PSUM"))
            pC = ctx.enter_context(tc.tile_pool(name="pC", bufs=2, space="PSUM"))

            xTr = xT.rearrange("(n p) m -> p n m", p=128)
            wqr = wq.rearrange("(n p) m -> p n m", p=128)
            wkvr = wkv.rearrange("(n p) m -> p n m", p=128)
            cstr = cst.rearrange("(n p) m -> p n m", p=128)

            wq_sb = wpool.tile([128, NDC, DQ], BF16, tag="wq")
            wkv_sb = wpool.tile([128, NDC, 2 * HD], BF16, tag="wkv")
            cst_sb = consts.tile([128, NST, 2 * HD], BF16, tag="cst")
            ident_sb = consts.tile([128, 128], BF16, tag="ident")
            qgb_sb = consts.tile([128, NQ], F32, tag="qgb")
            ones_sb = consts.tile([128, 128], BF16, tag="ones")
            msk_sb = consts.tile([128, 128], BF16, tag="msk")
            wo_sb = wpool.tile([128, NQ, D], BF16, tag="wo")

            x_blocks = [None] * NBLK
            x_blocks[0] = xpool.tile([128, NDC, 512], BF16, name="xb0",
                                     tag="xb")
            x_blocks[1] = xpool.tile([128, NDC, 512], BF16, name="xb1",
                                     tag="xb")

            # ---- tiny consts + first cst rows on the scalar DMA queue
            # (parallel issue; negligible bandwidth steal), everything else
            # priority-ordered on the sync queue.
            nc.scalar.dma_start(ident_sb[:], ident)
            nc.scalar.dma_start(qgb_sb[:], qgb)
            nc.scalar.dma_start(ones_sb[:], ones)
            nc.scalar.dma_start(msk_sb[:], msk)
            nc.scalar.dma_start(cst_sb[:, 0:4, :], cstr[:, 0:4, :])

            # sync queue: wq/xb0 interleaved 2-dc waves feed the dc-major
            # phase-A chains; then wkv, xb1, rest of cst, xb2/xb3 (issued
            # later), wo.
            for w in range(NDC // 2):
                nc.sync.dma_start(wq_sb[:, 2 * w:2 * w + 2, :],
                                  wqr[:, 2 * w:2 * w + 2, :])
                nc.sync.dma_start(x_blocks[0][:, 2 * w:2 * w + 2, :],
                                  xTr[:, 2 * w:2 * w + 2, 0:512])
            for w in range(4):
                nc.sync.dma_start(wkv_sb[:, 4 * w:4 * w + 4, :],
                                  wkvr[:, 4 * w:4 * w + 4, :])
            for w in range(4):
                nc.sync.dma_start(x_blocks[1][:, 4 * w:4 * w + 4, :],
                                  xTr[:, 4 * w:4 * w + 4, 512:1024])
            nc.sync.dma_start(cst_sb[:, 4:NST, :], cstr[:, 4:NST, :])

            # HAM warmup: PE work with NO input dependency (reads an
            # uninitialized SBUF scratch tile) so the clock gate is at 8/8
            # and the PE pipeline primed when the first real matmuls arrive.
            wsrc = consts.tile([128, 128], BF16, tag="wsrc")
            nc.vector.memset(wsrc[:], 1.0)
            warm = pC.tile([128, 512], F32, name="warm", tag="c")
            for i in range(55):
                nc.tensor.matmul(warm[:, 0:128], wsrc[:], wsrc[:],
                                 start=True, stop=True)

            qt_all = qt_pool.tile([128, NH1, S], BF16, name="qt_all",
                                  tag="qt_all")
            yt_tiles = [yt_pool.tile([128, S], BF16, name=f"yt{h}", tag=f"yt{h}")
                        for h in range(NQ)]
            v_tiles = [v_pool.tile([128, 132], BF16, name=f"v{st}", tag=f"v{st}")
                       for st in range(NST)]

            # ---- Phase 1: projections + rms-norm + rope + transpose ----
            # The PE transposes of s-tile st are deferred until the next
            # s-tile's projection matmuls have been emitted, so the rope
            # chain has a full tile of slack before the PE needs its output.
            tr_state = {"pend": []}

            def flush_one_tr():
                st, ro5 = tr_state["pend"].pop(0)
                bt = pB.tile([128, 1024], F32, name="bt", tag="b")
                ptv = bt[:, 0:NH1 * 64].bitcast(BF16)  # [128, 640] bf16
                for i in range(NH1):
                    nc.tensor.transpose(ptv[:, i * 128:(i + 1) * 128],
                                        ro5[:, i * HD:(i + 1) * HD],
                                        ident_sb[:])
                nc.scalar.copy(
                    qt_all[:, :, st * 128:(st + 1) * 128],
                    ptv.rearrange("p (h c) -> p h c", c=128))

            def flush_tr():
                while tr_state["pend"]:
                    flush_one_tr()

            def post_stile(st, pq, pkv):
                """Evacuate PSUM fast, rms stats, rsqrt (GpSimd), rope."""
                # evacuate PSUM on ScalarE/DVE (frees the accumulator banks
                # fast; the slower stats chain then runs from SBUF)
                qkv = sq_pool.tile([128, DQ + 2 * HD], BF16, tag="qkv")
                with tc.high_priority():
                    nc.scalar.copy(qkv[:, 0:DQ], pq)
                    nc.vector.tensor_copy(qkv[:, DQ:DQ + 2 * HD], pkv)

                # V row tile [v | 1] for the AV' matmuls
                nc.vector.tensor_copy(v_tiles[st][:, 0:HD],
                                      qkv[:, DQ + HD:DQ + 2 * HD])
                nc.vector.tensor_copy(v_tiles[st][:, HD:HD + 1],
                                      ones_sb[:, 0:1])

                # RMS stats: one batched square + one grouped reduce
                sq = sq_pool.tile([128, NH1 * HD], F32, tag="sq")
                nc.scalar.activation(sq[:], qkv[:, 0:DQ + HD], AF.Square)
                ssq = st_pool.tile([128, NH1], F32, tag="ssq")
                sq_g = AP(sq.tensor, sq.offset, [sq.ap[0], [HD, NH1], [1, HD]])
                nc.vector.tensor_reduce(ssq[:], sq_g, axis=AXL.X, op=ALU.add)

                # rinv = (mean(q^2)+eps)**-0.5: int bit-trick on DVE,
                # float Newton step on GpSimd (otherwise idle)
                m = st_pool.tile([128, NH1], F32, tag="m")
                nc.gpsimd.tensor_scalar(m[:], ssq[:], 1.0 / HD, RMS_EPS,
                                        op0=ALU.mult, op1=ALU.add)
                y0 = st_pool.tile([128, NH1], F32, tag="y0")
                nc.vector.tensor_scalar(y0[:].bitcast(I32),
                                        m[:].bitcast(I32), 1, None,
                                        op0=ALU.arith_shift_right)
                nc.vector.tensor_scalar(y0[:].bitcast(I32),
                                        y0[:].bitcast(I32),
                                        -1, 0x5F3759DF,
                                        op0=ALU.mult, op1=ALU.add)
                rinv = y0
                aa = st_pool.tile([128, NH1], F32, tag="nr_a")
                nc.gpsimd.tensor_mul(aa[:], rinv[:], rinv[:])
                nc.gpsimd.tensor_mul(aa[:], aa[:], m[:])
                nc.gpsimd.tensor_scalar(aa[:], aa[:], -0.5, 1.5,
                                        op0=ALU.mult, op1=ALU.add)
                nxt = st_pool.tile([128, NH1], F32, tag="nr_y")
                nc.gpsimd.tensor_mul(nxt[:], rinv[:], aa[:])
                rinv = nxt

                # rope straight from SBUF: per head [t|v] = (q*rinv)*[c|c|-s|s]
                cst_t = cst_sb[:, st, :]
                tv5 = tv_pool.tile([128, NH1 * 2 * HD], BF16, tag="tv5")
                for i in range(NH1):
                    q_ap = qkv[:, i * HD:(i + 1) * HD]
                    q_rep = AP(q_ap.tensor, q_ap.offset,
                               [q_ap.ap[0], [0, 2], [1, HD]])
                    nc.vector.scalar_tensor_tensor(
                        tv5[:, i * 2 * HD:(i + 1) * 2 * HD],
                        q_rep, rinv[:, i:i + 1], cst_t,
                        op0=ALU.mult, op1=ALU.mult)
                ro5 = ro_pool.tile([128, NH1 * HD], BF16, tag="ro5")
                b5 = tv5[:]
                t_view = AP(b5.tensor, b5.offset,
                            [b5.ap[0], [2 * HD, NH1], [H, 2], [1, H]])
                v_view = AP(b5.tensor, b5.offset + HD + H,
                            [b5.ap[0], [2 * HD, NH1], [-H, 2], [1, H]])
                r5 = ro5[:]
                o_view = AP(r5.tensor, r5.offset,
                            [r5.ap[0], [HD, NH1], [H, 2], [1, H]])
                nc.vector.tensor_add(o_view, t_view, v_view)
                tr_state["pend"].append((st, ro5))

            def process_stile(st, xb, st4):
                at = pA.tile([128, 1024], F32, name="at", tag="a")
                pq = at[:, 0:DQ]
                pkv = at[:, DQ:DQ + 2 * HD]
                for dc in range(NDC):
                    nc.tensor.matmul(pq, xb[:, dc, st4 * 128:(st4 + 1) * 128],
                                     wq_sb[:, dc, :],
                                     start=dc == 0, stop=dc == NDC - 1)
                for dc in range(NDC):
                    nc.tensor.matmul(pkv, xb[:, dc, st4 * 128:(st4 + 1) * 128],
                                     wkv_sb[:, dc, :],
                                     start=dc == 0, stop=dc == NDC - 1)
                if len(tr_state["pend"]) >= 2:
                    flush_one_tr()
                post_stile(st, pq, pkv)

            # ---- Phase A/B: block-0 s-tiles 0-3 dc-major across four
            # parallel PSUM accumulators, tracking the wq/xb0 DMA waves.
            pa0 = pA.tile([128, 1024], F32, name="pa0", tag="a")
            pa1 = pA.tile([128, 1024], F32, name="pa1", tag="a")
            pb0 = pB.tile([128, 1024], F32, name="pb0", tag="b")
            pc0 = pC.tile([128, 512], F32, name="pc0", tag="c")
            pc1 = pC.tile([128, 512], F32, name="pc1", tag="c")
            pqs = [pa0[:, 0:DQ], pa1[:, 0:DQ], pb0[:, 0:DQ], pc0[:, 0:DQ]]
            pkvs = [pa0[:, DQ:DQ + 2 * HD], pa1[:, DQ:DQ + 2 * HD],
                    pb0[:, DQ:DQ + 2 * HD], pc1[:, 0:2 * HD]]
            xb0 = x_blocks[0]
            for dc in range(NDC):
                for st4 in range(4):
                    nc.tensor.matmul(
                        pqs[st4], xb0[:, dc, st4 * 128:(st4 + 1) * 128],
                        wq_sb[:, dc, :], start=dc == 0, stop=dc == NDC - 1,
                        skip_group_check=True)
            # phase B: kv chains st-major with the posts woven in, so the
            # DVE/Scalar post work pipelines under the kv matmuls
            def kv_chain(st4):
                for dc in range(NDC):
                    nc.tensor.matmul(
                        pkvs[st4], xb0[:, dc, st4 * 128:(st4 + 1) * 128],
                        wkv_sb[:, dc, :], start=dc == 0, stop=dc == NDC - 1,
                        skip_group_check=True)
            kv_chain(0)
            kv_chain(1)
            post_stile(0, pqs[0], pkvs[0])
            kv_chain(2)
            post_stile(1, pqs[1], pkvs[1])
            kv_chain(3)
            post_stile(2, pqs[2], pkvs[2])
            post_stile(3, pqs[3], pkvs[3])

            # ---- Phases interleaved: projections block b -> attention
            # qc=b -> outproj qc=b-1 as PE filler between heads ----
            kt_row = qt_all[:, NQ, :]

            # out-projection dribbled one 128x512 tile at a time between
            # attention score groups (PE filler while ScalarE exps run);
            # outT DMA batched per 4 tiles.
            op_state = {"pending": [], "ob": None, "row": 0}

            def push_outproj(qcp):
                op_state["pending"].extend((qcp, dt) for dt in range(NST))

            def emit_outproj_unit(copy_eng="vector"):
                if not op_state["pending"]:
                    return
                qcp, dt = op_state["pending"].pop(0)
                if op_state["row"] == 0:
                    op_state["ob"] = ob_pool.tile([128, 4, 512], BF16,
                                                  name="ob", tag="ob")
                po = pC.tile([128, 512], F32, name="po", tag="c")
                for dqc in range(NQ):
                    nc.tensor.matmul(
                        po[:], wo_sb[:, dqc, dt * 128:(dt + 1) * 128],
                        yt_tiles[dqc][:, qcp * 512:(qcp + 1) * 512],
                        start=(dqc == 0), stop=(dqc == NQ - 1))
                if copy_eng == "scalar":
                    nc.scalar.copy(op_state["ob"][:, op_state["row"], :], po[:])
                else:
                    nc.vector.tensor_copy(
                        op_state["ob"][:, op_state["row"], :], po[:])
                op_state["row"] += 1
                if op_state["row"] == 4:
                    op_state["row"] = 0
                    nc.sync.dma_start(
                        outT[(dt - 3) * 128:(dt + 1) * 128,
                             qcp * 512:(qcp + 1) * 512].rearrange(
                                 "(n p) m -> p n m", p=128),
                        op_state["ob"][:])

            work_q = []  # pending s-tile closures (consumed as PE filler)

            def fill_slot():
                if work_q:
                    work_q.pop(0)()
                else:
                    emit_outproj_unit()

            def attention_head(qc, h):
                n_kt = 4 * qc + 4
                n_groups = n_kt // 2
                qs = qt_all[:, h, qc * 512:(qc + 1) * 512]
                sc_ap = qgb_sb[:, h:h + 1]

                def off_of(kt):
                    return max(0, kt - 4 * qc) * 128

                def emit_scores_group(g):
                    sp = pA.tile([128, 1024], F32, name="sp", tag="a")
                    ptp = pt_pool.tile([128, 2, 512], BF16, name="ptp",
                                       tag="ptp")
                    for u in (0, 1):
                        kt = 2 * g + u
                        off = off_of(kt)
                        nc.tensor.matmul(
                            sp[:, u * 512 + off:(u + 1) * 512],
                            kt_row[:, kt * 128:(kt + 1) * 128],
                            qs[:, off:512], start=True, stop=True)
                    if 2 * g + 1 < 4 * qc:  # both tiles non-diagonal
                        nc.scalar.activation(
                            ptp[:].rearrange("p a b -> p (a b)"),
                            sp[:], AF.Exp, scale=sc_ap)
                    else:
                        for u in (0, 1):
                            kt = 2 * g + u
                            off = off_of(kt)
                            nc.scalar.activation(ptp[:, u, off:512],
                                                 sp[:, u * 512 + off:(u + 1) * 512],
                                                 AF.Exp, scale=sc_ap)
                    for u in (0, 1):
                        kt = 2 * g + u
                        mdiag = kt - 4 * qc
                        if mdiag >= 0:
                            # only the 128-wide diagonal block needs masking
                            off = mdiag * 128
                            nc.vector.tensor_mul(
                                ptp[:, u, off:off + 128],
                                ptp[:, u, off:off + 128], msk_sb[:, 0:128])
                    return ptp

                def emit_av_group(g, ptp, pyp):
                    for u in (0, 1):
                        kt = 2 * g + u
                        j0 = max(0, kt - 4 * qc)
                        for j in range(j0, 4):
                            nc.tensor.matmul(
                                pyp[:, PYOFF[j]:PYOFF[j] + HD + 1],
                                ptp[:, u, j * 128:(j + 1) * 128],
                                v_tiles[kt][:, 0:HD + 1],
                                start=(kt == 0 and j in (0, 3)),
                                stop=(kt == 4 * qc + j),
                                skip_group_check=True)

                prev = emit_scores_group(0)
                fill_slot()
                # start=True on any matmul clears has_written for its whole
                # PSUM bank, which would wipe sibling accumulators sharing
                # the bank -- so zero the regions once and accumulate with
                # start=False throughout.
                pyp = pB.tile([128, 1024], F32, name="pyp", tag="b")
                for g in range(1, n_groups):
                    cur = emit_scores_group(g)
                    emit_av_group(g - 1, prev, pyp)
                    prev = cur
                    if qc < 2 or g % 2 == 1:
                        fill_slot()
                emit_av_group(n_groups - 1, prev, pyp)

                # normalize: rcp of the 4 denominator columns, then per-
                # partition scale of each [q,hd] block; transpose to [hd,q].
                rcp = rn_pool.tile([128, 4], F32, tag="rcp")
                r3 = AP(pyp.tensor, pyp.offset + HD,
                        [pyp.ap[0], [PYOFF[1] - PYOFF[0], 3]])
                nc.vector.reciprocal(out=rcp[:, 0:3], in_=r3)
                nc.vector.reciprocal(out=rcp[:, 3:4],
                                     in_=pyp[:, PYOFF[3] + HD:PYOFF[3] + HD + 1])
                yn = rn_pool.tile([128, NQ * HD], BF16, tag="yn")
                for j in range(4):
                    nc.vector.tensor_scalar(
                        yn[:, j * HD:(j + 1) * HD],
                        pyp[:, PYOFF[j]:PYOFF[j] + HD],
                        rcp[:, j:j + 1], None, op0=ALU.mult)
                ytr = pyp[:, 656:912].bitcast(BF16)  # [128, 512] bf16
                for j in range(4):
                    nc.tensor.transpose(ytr[:, j * 128:(j + 1) * 128],
                                        yn[:, j * HD:(j + 1) * HD],
                                        ident_sb[:])
                nc.vector.tensor_copy(
                    yt_tiles[h][:, qc * 512:(qc + 1) * 512], ytr)

            def prefetch_xb(b):
                x_blocks[b] = xpool.tile([128, NDC, 512], BF16,
                                         name=f"xb{b}", tag="xb")
                nc.sync.dma_start(x_blocks[b][:, 0:8, :],
                                  xTr[:, 0:8, b * 512:(b + 1) * 512])
                nc.sync.dma_start(x_blocks[b][:, 8:NDC, :],
                                  xTr[:, 8:NDC, b * 512:(b + 1) * 512])

            # stage A tail: s-tile 4 st-major (xb1 already streaming), so
            # the 1-deep transpose deferral never leaves a tile the next
            # stage's heads need pending at a stage boundary.
            process_stile(4, x_blocks[1], 0)

            # stages B..E: attention for qc in order; projection s-tiles of
            # later blocks and out-projection tiles serve as PE filler inside
            # the heads (supply matched to the heads' fill slots).
            STAGE_STILES = {0: range(5, 12), 1: range(12, 16),
                            2: range(0, 0), 3: range(0, 0)}
            for qc in range(NQC):
                if qc == 0:
                    prefetch_xb(2)
                    nc.sync.dma_start(
                        wo_sb[:], wo.rearrange("(n p) m -> p n m", p=128))
                    prefetch_xb(3)
                for st in STAGE_STILES[qc]:
                    work_q.append(
                        lambda st=st: process_stile(st, x_blocks[st // 4],
                                                    st % 4))
                flush_tr()  # heads of qc read qt up to st 4*qc+3
                for h in range(NQ):
                    attention_head(qc, h)
                while work_q:  # any unconsumed s-tiles must land before the
                    work_q.pop(0)()  # next stage's heads read their qt/v
                push_outproj(qc)
            di = 0
            while op_state["pending"]:
                emit_outproj_unit("scalar" if di % 2 else "vector")
                di += 1

    nc.compile()
    return nc


def get_nc():
    if "nc" not in _NC_CACHE:
        _NC_CACHE["nc"] = build_kernel()
    return _NC_CACHE["nc"]


def rope_tables(S=S, HD=HD):
    """Packed rope table [S, 2*HD]: [c | c | -s | s]."""
    f = 1.0 / (10000.0 ** (np.arange(0, HD, 2, dtype=np.float32) / HD))
    fr = np.outer(np.arange(S, dtype=np.float32), f)
    c = np.cos(fr).astype(np.float32)
    s = np.sin(fr).astype(np.float32)
    return np.concatenate([c, c, -s, s], axis=1)


def make_in_maps(x, Wq, Wk, Wv, Wo, qg):
    x = np.asarray(x, np.float32)
    Wq = np.asarray(Wq, np.float32)
    Wk = np.asarray(Wk, np.float32)
    Wv = np.asarray(Wv, np.float32)
    Wo = np.asarray(Wo, np.float32)
    qg = np.asarray(qg, np.float32)
    cst = rope_tables()
    ident = np.eye(128, dtype=BF16NP)
    ones = np.ones((128, 128), dtype=BF16NP)
    pp, ff = np.arange(128)[:, None], np.arange(128)[None, :]
    msk = (ff >= pp).astype(BF16NP)
    scale = np.float32(1.0 / np.sqrt(HD))
    xT = [np.ascontiguousarray(x[b].T).astype(BF16NP) for b in range(NB)]
    in_maps = []
    for b in range(NB):
        for tp in range(NTP):
            qgb_row = np.broadcast_to(
                (qg[tp * NQ:(tp + 1) * NQ] * scale)[None, :].astype(np.float32),
                (128, NQ)).copy()
            wkv = np.concatenate([
                Wk[tp * HD:(tp + 1) * HD, :].T,
                Wv[tp * HD:(tp + 1) * HD, :].T], axis=1)
            in_maps.append({
                "xT": xT[b],
                "wq": np.ascontiguousarray(
                    Wq[tp * DQ:(tp + 1) * DQ, :].T).astype(BF16NP),
                "wkv": np.ascontiguousarray(wkv).astype(BF16NP),
                "wo": np.ascontiguousarray(
                    Wo[:, tp * DQ:(tp + 1) * DQ].T).astype(BF16NP),
                "cst": cst.astype(BF16NP),
                "qgb": qgb_row,
                "ident": ident,
                "ones": ones,
                "msk": msk,
            })
    return in_maps


def run(x, Wq, Wk, Wv, Wo, qg, trace=False, **trace_kwargs):
    nc = get_nc()
    in_maps = make_in_maps(x, Wq, Wk, Wv, Wo, qg)
    res = run_bass_kernel_spmd(nc, in_maps, core_ids=list(range(NCORES)),
                               trace=trace, **trace_kwargs)
    out = np.empty((NB, S, D), np.float32)
    for b in range(NB):
        acc = res.results[b * NTP]["outT"].astype(np.float32)
        for tp in range(1, NTP):
            acc = acc + res.results[b * NTP + tp]["outT"].astype(np.float32)
        out[b] = acc.T
    return out, res


def kernel(x, Wq, Wk, Wv, Wo, qg):
    out, _ = run(x, Wq, Wk, Wv, Wo, qg)
    return out
